# revision 12
# baseline (speedup 1.0000x reference)
"""GNN message passing (src_mul_edge + segment_sum) on 8 Trainium2 cores. v2.

out[n] = sum_{e : dst[e]==n} e_att[e] * src_emb[src[e]]

Pull-mode, dst-sharded (disjoint outputs per core, no all-reduce):
  * Host groups edges by dst (CSR), lex-sorts nodes by (n_low, n_high) src-
    window edge counts so 128-node tiles have near-uniform slot counts, deals
    tiles onto cores by weight, and pads tile dims to a shared per-ordinal
    schedule so one compiled NEFF runs SPMD on all 8 cores.
  * Ordinals are batched in groups of G; each group's messages live in one
    SBUF buffer laid out [lo_0..lo_{G-1} | hi_0..hi_{G-1}] (slots x 64).
  * Device, per group: one dma_gather per src window (4 SWDGE queues round-
    robin so Q7 descriptor generation overlaps), one broadcast-attention
    multiply over the whole group, then per-tile reduces (lo + hi + add)
    and a [128, 64] output DMA.
  * dma_gather indices are int16 (max 32767) but N_SRC=50000: slots split
    into two windows of src_emb rows, [0, 32768) and [N_SRC-32768, N_SRC).
    Pad slots gather row 0 with attention 0.0.
"""

import numpy as np

N_SRC = 50000
N_DST = 50000
D = 64
N_CORES = 8
WINDOW = 32768
W2BASE = N_SRC - WINDOW  # 17232
LANES = 128
SLOT_BUDGET = 40

_cache: dict = {}

# test-harness knobs (ignored by the grading path)
TRACE = False
TRACE_DIR = None
LAST_EXEC_NS = None


def _group_layout(dlo_k, dhi_k):
    """Split ordinals into groups of ~SLOT_BUDGET slots; return per-group and
    per-ordinal absolute column offsets in the concatenated att/msg layout."""
    ntiles = len(dlo_k)
    bounds = [0]
    acc = 0
    for k in range(ntiles):
        d = dlo_k[k] + dhi_k[k]
        if acc > 0 and acc + d > SLOT_BUDGET:
            bounds.append(k)
            acc = 0
        acc += d
    bounds.append(ntiles)
    groups = []
    goff = 0
    loff_abs = [0] * ntiles   # att-col offset of ordinal k's lo region
    hoff_abs = [0] * ntiles   # att-col offset of ordinal k's hi region
    for b0, b1 in zip(bounds[:-1], bounds[1:]):
        ks = list(range(b0, b1))
        glo = sum(dlo_k[k] for k in ks)
        ghi = sum(dhi_k[k] for k in ks)
        o = 0
        for k in ks:
            loff_abs[k] = goff + o
            o += dlo_k[k]
        o = 0
        for k in ks:
            hoff_abs[k] = goff + glo + o
            o += dhi_k[k]
        groups.append({"ks": ks, "glo": glo, "ghi": ghi, "goff": goff})
        goff += glo + ghi
    return groups, loff_abs, hoff_abs, goff


def _build_nc(dlo_k, dhi_k, nlo_total, nhi_total, n_out_rows):
    import concourse.bacc as bacc
    import concourse.mybir as mybir
    from concourse.tile import TileContext
    from concourse.library_config import mlp

    groups, loff_abs, hoff_abs, na_total = _group_layout(dlo_k, dhi_k)
    gmax = max(g["glo"] + g["ghi"] for g in groups)

    nc = bacc.Bacc(
        "TRN2", target_bir_lowering=False, debug=False, num_swdge_queues=4
    )
    emb = nc.dram_tensor("emb", [N_SRC, D], mybir.dt.float32, kind="ExternalInput")
    att = nc.dram_tensor("att", [LANES, na_total], mybir.dt.float32, kind="ExternalInput")
    ilo = nc.dram_tensor("ilo", [LANES, max(nlo_total, 1)], mybir.dt.int16, kind="ExternalInput")
    ihi = nc.dram_tensor("ihi", [LANES, max(nhi_total, 1)], mybir.dt.int16, kind="ExternalInput")
    out = nc.dram_tensor("out", [n_out_rows, D], mybir.dt.float32, kind="ExternalOutput")

    with TileContext(nc) as tc:
        nc.gpsimd.load_library(mlp)
        with (
            tc.tile_pool(name="msg", bufs=12) as msg_pool,
            tc.tile_pool(name="meta", bufs=1) as meta_pool,
            tc.tile_pool(name="acc", bufs=12) as acc_pool,
        ):
            att_all = meta_pool.tile([LANES, na_total], mybir.dt.float32, tag="att")
            ilo_all = meta_pool.tile([LANES, max(nlo_total, 1)], mybir.dt.int16, tag="ilo")
            ihi_all = meta_pool.tile([LANES, max(nhi_total, 1)], mybir.dt.int16, tag="ihi")
            nc.sync.dma_start(att_all[:], att[:])
            nc.sync.dma_start(ilo_all[:], ilo[:])
            nc.sync.dma_start(ihi_all[:], ihi[:])
            loff = 0
            hoff = 0
            qrot = 0
            for g in groups:
                glo, ghi = g["glo"], g["ghi"]
                gt = glo + ghi
                if gt == 0:
                    continue
                msg_t = msg_pool.tile([LANES, gmax, D], mybir.dt.float32, tag="msg")
                if glo > 0:
                    nc.gpsimd.dma_gather(
                        msg_t[:, 0:glo, :],
                        emb[0:WINDOW, :],
                        ilo_all[:, loff : loff + 8 * glo],
                        glo * LANES,
                        glo * LANES,
                        D,
                        single_packet=False,
                        queue_num=qrot % 4,
                    )
                    qrot += 1
                if ghi > 0:
                    nc.gpsimd.dma_gather(
                        msg_t[:, glo:gt, :],
                        emb[W2BASE:N_SRC, :],
                        ihi_all[:, hoff : hoff + 8 * ghi],
                        ghi * LANES,
                        ghi * LANES,
                        D,
                        single_packet=False,
                        queue_num=qrot % 4,
                    )
                    qrot += 1
                att_b = (
                    att_all[:, g["goff"] : g["goff"] + gt]
                    .unsqueeze(2)
                    .broadcast_to([LANES, gt, D])
                )
                nc.vector.tensor_tensor(
                    msg_t[:, :gt, :], msg_t[:, :gt, :], att_b, mybir.AluOpType.mult
                )
                for k in g["ks"]:
                    dlo, dhi = dlo_k[k], dhi_k[k]
                    if dlo + dhi == 0:
                        continue
                    lo0 = loff_abs[k] - g["goff"]
                    hi0 = hoff_abs[k] - g["goff"]
                    acc_t = acc_pool.tile([LANES, D], mybir.dt.float32, tag="acc")
                    if dlo > 0 and dhi > 0:
                        accb_t = acc_pool.tile([LANES, D], mybir.dt.float32, tag="accb")
                        nc.vector.tensor_reduce(
                            acc_t[:],
                            msg_t[:, lo0 : lo0 + dlo, :].transpose([0, 2, 1]),
                            axis=mybir.AxisListType.X,
                            op=mybir.AluOpType.add,
                        )
                        nc.vector.tensor_reduce(
                            accb_t[:],
                            msg_t[:, hi0 : hi0 + dhi, :].transpose([0, 2, 1]),
                            axis=mybir.AxisListType.X,
                            op=mybir.AluOpType.add,
                        )
                        nc.vector.tensor_tensor(
                            acc_t[:], acc_t[:], accb_t[:], mybir.AluOpType.add
                        )
                    elif dlo > 0:
                        nc.vector.tensor_reduce(
                            acc_t[:],
                            msg_t[:, lo0 : lo0 + dlo, :].transpose([0, 2, 1]),
                            axis=mybir.AxisListType.X,
                            op=mybir.AluOpType.add,
                        )
                    else:
                        nc.vector.tensor_reduce(
                            acc_t[:],
                            msg_t[:, hi0 : hi0 + dhi, :].transpose([0, 2, 1]),
                            axis=mybir.AxisListType.X,
                            op=mybir.AluOpType.add,
                        )
                    nc.sync.dma_start(out[k * LANES : (k + 1) * LANES, :], acc_t[:])
                loff += 8 * glo
                hoff += 8 * ghi
    nc.compile()
    return nc


def _wrap_idx(idx_flat):
    """[n] int16 position-ordered -> [128, n//16] wrapped+replicated tile."""
    w = idx_flat.reshape(-1, 16).T  # [16, n/16]
    return np.tile(w, (8, 1))


def plan_and_build(src_idx, dst_idx, e_att, n_src=N_SRC, n_dst=N_DST,
                   n_cores=N_CORES, window=WINDOW, w2base=W2BASE):
    """Host-side planning. Returns per-core input arrays + metadata."""
    E = src_idx.shape[0]
    att_flat = np.asarray(e_att, dtype=np.float32).reshape(-1)
    src_idx = np.asarray(src_idx, dtype=np.int64)
    dst_idx = np.asarray(dst_idx, dtype=np.int64)

    deg = np.bincount(dst_idx, minlength=n_dst)
    is_high = src_idx >= window
    nlow = np.bincount(dst_idx[~is_high], minlength=n_dst)
    nhigh = deg - nlow

    nodeorder = np.lexsort((nhigh, nlow))  # ascending by (nlow, nhigh)
    tiles_per_core = -(-n_dst // (LANES * n_cores))
    nodes_pad = LANES * tiles_per_core * n_cores
    n_tiles = nodes_pad // LANES

    pos = np.empty(n_dst, dtype=np.int64)
    pos[nodeorder] = np.arange(n_dst)

    nlow_s = np.zeros(nodes_pad, dtype=np.int64)
    nhigh_s = np.zeros(nodes_pad, dtype=np.int64)
    nlow_s[: n_dst] = nlow[nodeorder]
    nhigh_s[: n_dst] = nhigh[nodeorder]
    dlo_tile = nlow_s.reshape(n_tiles, LANES).max(axis=1)
    dhi_tile = nhigh_s.reshape(n_tiles, LANES).max(axis=1)

    w = dlo_tile + dhi_tile
    tile_rank = np.argsort(-w, kind="stable")
    T = tile_rank.reshape(tiles_per_core, n_cores)  # [ordinal, core]
    dlo_k = dlo_tile[T].max(axis=1)  # [ordinal]
    dhi_k = dhi_tile[T].max(axis=1)

    ord_of_tile = np.empty(n_tiles, dtype=np.int64)
    core_of_tile = np.empty(n_tiles, dtype=np.int64)
    for k in range(tiles_per_core):
        for c in range(n_cores):
            ord_of_tile[T[k, c]] = k
            core_of_tile[T[k, c]] = c

    groups, loff_abs, hoff_abs, na_total = _group_layout(
        tuple(int(x) for x in dlo_k), tuple(int(x) for x in dhi_k)
    )
    loff_abs = np.asarray(loff_abs, dtype=np.int64)
    hoff_abs = np.asarray(hoff_abs, dtype=np.int64)
    dlo_sum = int(dlo_k.sum())
    dhi_sum = int(dhi_k.sum())

    # per-edge placement
    t_e = pos[dst_idx] // LANES
    lane_e = pos[dst_idx] % LANES
    k_e = ord_of_tile[t_e]
    c_e = core_of_tile[t_e]

    # rank within node, low edges first
    eorder = np.lexsort((is_high, dst_idx))
    starts = np.concatenate([[0], np.cumsum(deg)])
    rank_sorted = np.arange(E) - starts[dst_idx[eorder]]
    rank = np.empty(E, dtype=np.int64)
    rank[eorder] = rank_sorted

    # absolute att column for each edge (group layout)
    att_col = np.where(
        is_high,
        hoff_abs[k_e] + rank - nlow[dst_idx],
        loff_abs[k_e] + rank,
    )

    att3 = np.zeros((n_cores, LANES, na_total), dtype=np.float32)
    att3[c_e, lane_e, att_col] = att_flat

    # index arrays, slot-major per ordinal: [n_cores, dlo_sum, 128]
    iloff_k = np.concatenate([[0], np.cumsum(dlo_k)])[:-1]
    ihoff_k = np.concatenate([[0], np.cumsum(dhi_k)])[:-1]
    ilo3 = np.zeros((n_cores, max(dlo_sum, 1), LANES), dtype=np.int16)
    ihi3 = np.zeros((n_cores, max(dhi_sum, 1), LANES), dtype=np.int16)
    lo_m = ~is_high
    ilo3[c_e[lo_m], iloff_k[k_e[lo_m]] + rank[lo_m], lane_e[lo_m]] = src_idx[lo_m].astype(np.int16)
    hi_m = is_high
    ihi3[c_e[hi_m], ihoff_k[k_e[hi_m]] + (rank[hi_m] - nlow[dst_idx[hi_m]]), lane_e[hi_m]] = (
        src_idx[hi_m] - w2base
    ).astype(np.int16)

    # wrap idx arrays per ordinal into the [128, 8*D] device layout
    ilo_cores = []
    ihi_cores = []
    ntiles = len(dlo_k)
    for c in range(n_cores):
        lo_parts = [np.zeros((LANES, 0), dtype=np.int16)]
        hi_parts = [np.zeros((LANES, 0), dtype=np.int16)]
        for k in range(ntiles):
            if dlo_k[k] > 0:
                lo_parts.append(
                    _wrap_idx(ilo3[c, iloff_k[k] : iloff_k[k] + dlo_k[k], :].ravel())
                )
            if dhi_k[k] > 0:
                hi_parts.append(
                    _wrap_idx(ihi3[c, ihoff_k[k] : ihoff_k[k] + dhi_k[k], :].ravel())
                )
        lo_cat = np.concatenate(lo_parts, axis=1) if len(lo_parts) > 1 else np.zeros((LANES, 1), np.int16)
        hi_cat = np.concatenate(hi_parts, axis=1) if len(hi_parts) > 1 else np.zeros((LANES, 1), np.int16)
        ilo_cores.append(np.ascontiguousarray(lo_cat))
        ihi_cores.append(np.ascontiguousarray(hi_cat))

    # node id at (core, ordinal, lane) for un-permuting
    node_map = np.full((n_cores, tiles_per_core * LANES), -1, dtype=np.int64)
    sorted_ids = np.full(nodes_pad, -1, dtype=np.int64)
    sorted_ids[: n_dst] = nodeorder
    for c in range(n_cores):
        for k in range(tiles_per_core):
            t = T[k, c]
            node_map[c, k * LANES : (k + 1) * LANES] = sorted_ids[t * LANES : (t + 1) * LANES]

    return {
        "dlo_k": tuple(int(x) for x in dlo_k),
        "dhi_k": tuple(int(x) for x in dhi_k),
        "na_total": na_total,
        "nlo_total": ilo_cores[0].shape[1],
        "nhi_total": ihi_cores[0].shape[1],
        "n_out_rows": tiles_per_core * LANES,
        "att3": att3,
        "ilo_cores": ilo_cores,
        "ihi_cores": ihi_cores,
        "node_map": node_map,
        "tiles_per_core": tiles_per_core,
    }


def kernel(src_emb, e_att, src_idx, dst_idx):
    from concourse.bass_utils import run_bass_kernel_spmd

    src_emb = np.asarray(src_emb, dtype=np.float32)
    plan = plan_and_build(np.asarray(src_idx), np.asarray(dst_idx), np.asarray(e_att))

    key = (plan["dlo_k"], plan["dhi_k"], plan["nlo_total"], plan["nhi_total"])
    if key not in _cache:
        _cache.clear()
        _cache[key] = _build_nc(
            plan["dlo_k"], plan["dhi_k"],
            plan["nlo_total"], plan["nhi_total"], plan["n_out_rows"],
        )
    nc = _cache[key]

    in_maps = []
    for c in range(N_CORES):
        in_maps.append(
            {
                "emb": src_emb,
                "att": plan["att3"][c],
                "ilo": plan["ilo_cores"][c],
                "ihi": plan["ihi_cores"][c],
            }
        )
    kwargs = {}
    if TRACE:
        kwargs = {"trace": True, "tmpdir": TRACE_DIR}
    res = run_bass_kernel_spmd(nc, in_maps, core_ids=list(range(N_CORES)), **kwargs)
    global LAST_EXEC_NS
    LAST_EXEC_NS = res.exec_time_ns

    out_full = np.zeros((N_DST, D), dtype=np.float32)
    for c in range(N_CORES):
        ids = plan["node_map"][c]
        valid = ids >= 0
        out_full[ids[valid]] = res.results[c]["out"][valid]
    return out_full


# revision 13
# speedup vs baseline: 1.1030x; 1.1030x over previous
"""GNN message passing (src_mul_edge + segment_sum) on 8 Trainium2 cores. v2.

out[n] = sum_{e : dst[e]==n} e_att[e] * src_emb[src[e]]

Pull-mode, dst-sharded (disjoint outputs per core, no all-reduce):
  * Host groups edges by dst (CSR), lex-sorts nodes by (n_low, n_high) src-
    window edge counts so 128-node tiles have near-uniform slot counts, deals
    tiles onto cores by weight, and pads tile dims to a shared per-ordinal
    schedule so one compiled NEFF runs SPMD on all 8 cores.
  * Ordinals are batched in groups of G; each group's messages live in one
    SBUF buffer laid out [lo_0..lo_{G-1} | hi_0..hi_{G-1}] (slots x 64).
  * Device, per group: one dma_gather per src window (4 SWDGE queues round-
    robin so Q7 descriptor generation overlaps), one broadcast-attention
    multiply over the whole group, then per-tile reduces (lo + hi + add)
    and a [128, 64] output DMA.
  * dma_gather indices are int16 (max 32767) but N_SRC=50000: slots split
    into two windows of src_emb rows, [0, 32768) and [N_SRC-32768, N_SRC).
    Pad slots gather row 0 with attention 0.0.
"""

import numpy as np

N_SRC = 50000
N_DST = 50000
D = 64
N_CORES = 8
WINDOW = 32768
W2BASE = N_SRC - WINDOW  # 17232
LANES = 128
SLOT_BUDGET = 64

_cache: dict = {}

# test-harness knobs (ignored by the grading path)
TRACE = False
TRACE_DIR = None
LAST_EXEC_NS = None


def _group_layout(dlo_k, dhi_k):
    """Split ordinals into groups of ~SLOT_BUDGET slots; return per-group and
    per-ordinal absolute column offsets in the concatenated att/msg layout."""
    ntiles = len(dlo_k)
    bounds = [0]
    acc = 0
    for k in range(ntiles):
        d = dlo_k[k] + dhi_k[k]
        if acc > 0 and acc + d > SLOT_BUDGET:
            bounds.append(k)
            acc = 0
        acc += d
    bounds.append(ntiles)
    groups = []
    goff = 0
    loff_abs = [0] * ntiles   # att-col offset of ordinal k's lo region
    hoff_abs = [0] * ntiles   # att-col offset of ordinal k's hi region
    for b0, b1 in zip(bounds[:-1], bounds[1:]):
        ks = list(range(b0, b1))
        glo = sum(dlo_k[k] for k in ks)
        ghi = sum(dhi_k[k] for k in ks)
        o = 0
        for k in ks:
            loff_abs[k] = goff + o
            o += dlo_k[k]
        o = 0
        for k in ks:
            hoff_abs[k] = goff + glo + o
            o += dhi_k[k]
        groups.append({"ks": ks, "glo": glo, "ghi": ghi, "goff": goff})
        goff += glo + ghi
    return groups, loff_abs, hoff_abs, goff


def _build_nc(dlo_k, dhi_k, nlo_total, nhi_total, n_out_rows):
    import concourse.bacc as bacc
    import concourse.mybir as mybir
    from concourse.tile import TileContext
    from concourse.library_config import mlp

    groups, loff_abs, hoff_abs, na_total = _group_layout(dlo_k, dhi_k)
    gmax = max(g["glo"] + g["ghi"] for g in groups)

    nc = bacc.Bacc(
        "TRN2", target_bir_lowering=False, debug=False, num_swdge_queues=4
    )
    emb = nc.dram_tensor("emb", [N_SRC, D], mybir.dt.float32, kind="ExternalInput")
    att = nc.dram_tensor("att", [LANES, na_total], mybir.dt.float32, kind="ExternalInput")
    ilo = nc.dram_tensor("ilo", [LANES, max(nlo_total, 1)], mybir.dt.int16, kind="ExternalInput")
    ihi = nc.dram_tensor("ihi", [LANES, max(nhi_total, 1)], mybir.dt.int16, kind="ExternalInput")
    out = nc.dram_tensor("out", [n_out_rows, D], mybir.dt.float32, kind="ExternalOutput")

    with TileContext(nc) as tc:
        nc.gpsimd.load_library(mlp)
        with (
            tc.tile_pool(name="msg", bufs=10) as msg_pool,
            tc.tile_pool(name="meta", bufs=1) as meta_pool,
            tc.tile_pool(name="acc", bufs=12) as acc_pool,
        ):
            att_all = meta_pool.tile([LANES, na_total], mybir.dt.float32, tag="att")
            ilo_all = meta_pool.tile([LANES, max(nlo_total, 1)], mybir.dt.int16, tag="ilo")
            ihi_all = meta_pool.tile([LANES, max(nhi_total, 1)], mybir.dt.int16, tag="ihi")
            nc.sync.dma_start(att_all[:], att[:])
            nc.sync.dma_start(ilo_all[:], ilo[:])
            nc.sync.dma_start(ihi_all[:], ihi[:])
            loff = 0
            hoff = 0
            qrot = 0
            for g in groups:
                glo, ghi = g["glo"], g["ghi"]
                gt = glo + ghi
                if gt == 0:
                    continue
                msg_t = msg_pool.tile([LANES, gmax, D], mybir.dt.float32, tag="msg")
                if glo > 0:
                    nc.gpsimd.dma_gather(
                        msg_t[:, 0:glo, :],
                        emb[0:WINDOW, :],
                        ilo_all[:, loff : loff + 8 * glo],
                        glo * LANES,
                        glo * LANES,
                        D,
                        single_packet=False,
                        queue_num=qrot % 4,
                    )
                    qrot += 1
                if ghi > 0:
                    nc.gpsimd.dma_gather(
                        msg_t[:, glo:gt, :],
                        emb[W2BASE:N_SRC, :],
                        ihi_all[:, hoff : hoff + 8 * ghi],
                        ghi * LANES,
                        ghi * LANES,
                        D,
                        single_packet=False,
                        queue_num=qrot % 4,
                    )
                    qrot += 1
                att_b = (
                    att_all[:, g["goff"] : g["goff"] + gt]
                    .unsqueeze(2)
                    .broadcast_to([LANES, gt, D])
                )
                nc.vector.tensor_tensor(
                    msg_t[:, :gt, :], msg_t[:, :gt, :], att_b, mybir.AluOpType.mult
                )
                for k in g["ks"]:
                    dlo, dhi = dlo_k[k], dhi_k[k]
                    if dlo + dhi == 0:
                        continue
                    lo0 = loff_abs[k] - g["goff"]
                    hi0 = hoff_abs[k] - g["goff"]
                    acc_t = acc_pool.tile([LANES, D], mybir.dt.float32, tag="acc")
                    if dlo > 0 and dhi > 0:
                        accb_t = acc_pool.tile([LANES, D], mybir.dt.float32, tag="accb")
                        nc.vector.tensor_reduce(
                            acc_t[:],
                            msg_t[:, lo0 : lo0 + dlo, :].transpose([0, 2, 1]),
                            axis=mybir.AxisListType.X,
                            op=mybir.AluOpType.add,
                        )
                        nc.vector.tensor_reduce(
                            accb_t[:],
                            msg_t[:, hi0 : hi0 + dhi, :].transpose([0, 2, 1]),
                            axis=mybir.AxisListType.X,
                            op=mybir.AluOpType.add,
                        )
                        nc.vector.tensor_tensor(
                            acc_t[:], acc_t[:], accb_t[:], mybir.AluOpType.add
                        )
                    elif dlo > 0:
                        nc.vector.tensor_reduce(
                            acc_t[:],
                            msg_t[:, lo0 : lo0 + dlo, :].transpose([0, 2, 1]),
                            axis=mybir.AxisListType.X,
                            op=mybir.AluOpType.add,
                        )
                    else:
                        nc.vector.tensor_reduce(
                            acc_t[:],
                            msg_t[:, hi0 : hi0 + dhi, :].transpose([0, 2, 1]),
                            axis=mybir.AxisListType.X,
                            op=mybir.AluOpType.add,
                        )
                    nc.sync.dma_start(out[k * LANES : (k + 1) * LANES, :], acc_t[:])
                loff += 8 * glo
                hoff += 8 * ghi
    nc.compile()
    return nc


def _wrap_idx(idx_flat):
    """[n] int16 position-ordered -> [128, n//16] wrapped+replicated tile."""
    w = idx_flat.reshape(-1, 16).T  # [16, n/16]
    return np.tile(w, (8, 1))


def plan_and_build(src_idx, dst_idx, e_att, n_src=N_SRC, n_dst=N_DST,
                   n_cores=N_CORES, window=WINDOW, w2base=W2BASE):
    """Host-side planning. Returns per-core input arrays + metadata."""
    E = src_idx.shape[0]
    att_flat = np.asarray(e_att, dtype=np.float32).reshape(-1)
    src_idx = np.asarray(src_idx, dtype=np.int64)
    dst_idx = np.asarray(dst_idx, dtype=np.int64)

    deg = np.bincount(dst_idx, minlength=n_dst)
    is_high = src_idx >= window
    nlow = np.bincount(dst_idx[~is_high], minlength=n_dst)
    nhigh = deg - nlow

    nodeorder = np.lexsort((nhigh, nlow))  # ascending by (nlow, nhigh)
    tiles_per_core = -(-n_dst // (LANES * n_cores))
    nodes_pad = LANES * tiles_per_core * n_cores
    n_tiles = nodes_pad // LANES

    pos = np.empty(n_dst, dtype=np.int64)
    pos[nodeorder] = np.arange(n_dst)

    nlow_s = np.zeros(nodes_pad, dtype=np.int64)
    nhigh_s = np.zeros(nodes_pad, dtype=np.int64)
    nlow_s[: n_dst] = nlow[nodeorder]
    nhigh_s[: n_dst] = nhigh[nodeorder]
    dlo_tile = nlow_s.reshape(n_tiles, LANES).max(axis=1)
    dhi_tile = nhigh_s.reshape(n_tiles, LANES).max(axis=1)

    w = dlo_tile + dhi_tile
    tile_rank = np.argsort(-w, kind="stable")
    T = tile_rank.reshape(tiles_per_core, n_cores)  # [ordinal, core]
    dlo_k = dlo_tile[T].max(axis=1)  # [ordinal]
    dhi_k = dhi_tile[T].max(axis=1)

    ord_of_tile = np.empty(n_tiles, dtype=np.int64)
    core_of_tile = np.empty(n_tiles, dtype=np.int64)
    for k in range(tiles_per_core):
        for c in range(n_cores):
            ord_of_tile[T[k, c]] = k
            core_of_tile[T[k, c]] = c

    groups, loff_abs, hoff_abs, na_total = _group_layout(
        tuple(int(x) for x in dlo_k), tuple(int(x) for x in dhi_k)
    )
    loff_abs = np.asarray(loff_abs, dtype=np.int64)
    hoff_abs = np.asarray(hoff_abs, dtype=np.int64)
    dlo_sum = int(dlo_k.sum())
    dhi_sum = int(dhi_k.sum())

    # per-edge placement
    t_e = pos[dst_idx] // LANES
    lane_e = pos[dst_idx] % LANES
    k_e = ord_of_tile[t_e]
    c_e = core_of_tile[t_e]

    # rank within node, low edges first
    eorder = np.lexsort((is_high, dst_idx))
    starts = np.concatenate([[0], np.cumsum(deg)])
    rank_sorted = np.arange(E) - starts[dst_idx[eorder]]
    rank = np.empty(E, dtype=np.int64)
    rank[eorder] = rank_sorted

    # absolute att column for each edge (group layout)
    att_col = np.where(
        is_high,
        hoff_abs[k_e] + rank - nlow[dst_idx],
        loff_abs[k_e] + rank,
    )

    att3 = np.zeros((n_cores, LANES, na_total), dtype=np.float32)
    att3[c_e, lane_e, att_col] = att_flat

    # index arrays, slot-major per ordinal: [n_cores, dlo_sum, 128]
    iloff_k = np.concatenate([[0], np.cumsum(dlo_k)])[:-1]
    ihoff_k = np.concatenate([[0], np.cumsum(dhi_k)])[:-1]
    ilo3 = np.zeros((n_cores, max(dlo_sum, 1), LANES), dtype=np.int16)
    ihi3 = np.zeros((n_cores, max(dhi_sum, 1), LANES), dtype=np.int16)
    lo_m = ~is_high
    ilo3[c_e[lo_m], iloff_k[k_e[lo_m]] + rank[lo_m], lane_e[lo_m]] = src_idx[lo_m].astype(np.int16)
    hi_m = is_high
    ihi3[c_e[hi_m], ihoff_k[k_e[hi_m]] + (rank[hi_m] - nlow[dst_idx[hi_m]]), lane_e[hi_m]] = (
        src_idx[hi_m] - w2base
    ).astype(np.int16)

    # wrap idx arrays per ordinal into the [128, 8*D] device layout
    ilo_cores = []
    ihi_cores = []
    ntiles = len(dlo_k)
    for c in range(n_cores):
        lo_parts = [np.zeros((LANES, 0), dtype=np.int16)]
        hi_parts = [np.zeros((LANES, 0), dtype=np.int16)]
        for k in range(ntiles):
            if dlo_k[k] > 0:
                lo_parts.append(
                    _wrap_idx(ilo3[c, iloff_k[k] : iloff_k[k] + dlo_k[k], :].ravel())
                )
            if dhi_k[k] > 0:
                hi_parts.append(
                    _wrap_idx(ihi3[c, ihoff_k[k] : ihoff_k[k] + dhi_k[k], :].ravel())
                )
        lo_cat = np.concatenate(lo_parts, axis=1) if len(lo_parts) > 1 else np.zeros((LANES, 1), np.int16)
        hi_cat = np.concatenate(hi_parts, axis=1) if len(hi_parts) > 1 else np.zeros((LANES, 1), np.int16)
        ilo_cores.append(np.ascontiguousarray(lo_cat))
        ihi_cores.append(np.ascontiguousarray(hi_cat))

    # node id at (core, ordinal, lane) for un-permuting
    node_map = np.full((n_cores, tiles_per_core * LANES), -1, dtype=np.int64)
    sorted_ids = np.full(nodes_pad, -1, dtype=np.int64)
    sorted_ids[: n_dst] = nodeorder
    for c in range(n_cores):
        for k in range(tiles_per_core):
            t = T[k, c]
            node_map[c, k * LANES : (k + 1) * LANES] = sorted_ids[t * LANES : (t + 1) * LANES]

    return {
        "dlo_k": tuple(int(x) for x in dlo_k),
        "dhi_k": tuple(int(x) for x in dhi_k),
        "na_total": na_total,
        "nlo_total": ilo_cores[0].shape[1],
        "nhi_total": ihi_cores[0].shape[1],
        "n_out_rows": tiles_per_core * LANES,
        "att3": att3,
        "ilo_cores": ilo_cores,
        "ihi_cores": ihi_cores,
        "node_map": node_map,
        "tiles_per_core": tiles_per_core,
    }


def kernel(src_emb, e_att, src_idx, dst_idx):
    from concourse.bass_utils import run_bass_kernel_spmd

    src_emb = np.asarray(src_emb, dtype=np.float32)
    plan = plan_and_build(np.asarray(src_idx), np.asarray(dst_idx), np.asarray(e_att))

    key = (plan["dlo_k"], plan["dhi_k"], plan["nlo_total"], plan["nhi_total"])
    if key not in _cache:
        _cache.clear()
        _cache[key] = _build_nc(
            plan["dlo_k"], plan["dhi_k"],
            plan["nlo_total"], plan["nhi_total"], plan["n_out_rows"],
        )
    nc = _cache[key]

    in_maps = []
    for c in range(N_CORES):
        in_maps.append(
            {
                "emb": src_emb,
                "att": plan["att3"][c],
                "ilo": plan["ilo_cores"][c],
                "ihi": plan["ihi_cores"][c],
            }
        )
    kwargs = {}
    if TRACE:
        kwargs = {"trace": True, "tmpdir": TRACE_DIR}
    res = run_bass_kernel_spmd(nc, in_maps, core_ids=list(range(N_CORES)), **kwargs)
    global LAST_EXEC_NS
    LAST_EXEC_NS = res.exec_time_ns

    out_full = np.zeros((N_DST, D), dtype=np.float32)
    for c in range(N_CORES):
        ids = plan["node_map"][c]
        valid = ids >= 0
        out_full[ids[valid]] = res.results[c]["out"][valid]
    return out_full
